# revision 60
# baseline (speedup 1.0000x reference)
"""Trainium2 Bass kernel for nn_ChannelSpatialModulatedConv2d.

Data-parallel over batch across 8 NeuronCores (4 samples each).

All batch-parallel small math (EqualLinear style, weight demod, spatial map
+ its demod) runs on HOST in float64 — a few MFLOPs. The per-sample
modulated+demodulated weights are also folded on host:

  wsc[b, ci, co*kk] = weight[co,ci,kk] * style[b,ci] * (CS*demod[b,co]*demod_sp[b])

so the device kernel is a pure bf16 conv stream:

  per (b, m, n) output tile [128co x 512yx]: 18 accumulating bf16 matmuls
     (2 ci-tiles x 9 taps) over a zero-padded 66x66 image, shifted-window APs
  epilogue (DVE): out = psum * spmap[yx]   (spmap = raw sp map, bf16,
     broadcast 1->128 partitions by DMA)

PE stream = ~40 warm-up matmuls (HAM K=8/8 by the time real MMs start)
followed by 1152 conv matmuls at the N=512 streaming roofline. The only
other device compute is the 64 epilogue multiplies on DVE.

DMA: sync(HWDGE) = x bands + outs (first x band thinned to rows 0-10);
scalar(HWDGE) = wsc tiles (sample 0 split into m-halves so conv starts
ASAP) + spmap broadcasts (sample 0 up-front, rest staggered per sample).

The baked walrus build only supports ONE sync wait per instruction, so the
Bass subclass rewrites the scheduled BIR JSON, hoisting extra waits onto
single-wait EventSemaphore carriers inserted before the instruction (same
engine => identical blocking semantics).
"""

import json
import sys
from contextlib import ExitStack

for _p in ("/opt/pypackages", "/opt/trn_rl_repo"):
    if _p not in sys.path:
        sys.path.insert(0, _p)

import ml_dtypes
import numpy as np

import concourse.bass as bass
import concourse.mybir as mybir
import concourse.tile as tile
from concourse.bass_utils import run_bass_kernel_spmd

# Problem constants (hardcoded per harness contract)
B, CIN, COUT, K = 32, 256, 256, 3
STYLE_DIM, SP = 512, 64
EPS = 1e-6
LS = 1.0 / (STYLE_DIM // 2) ** 0.5      # EqualLinear scale = 1/16
CS = 1.0 / (CIN * K * K) ** 0.5         # conv fan-in scale = 1/48
N_CORES = 8
BPC = B // N_CORES                      # samples per core = 4
SPP = SP + 2                            # padded image dim = 66
CKK = COUT * K * K                      # 2304 free columns in weight layout
YX = SP * SP                            # 4096 spatial positions
HALF = (CKK // 2)                       # m=0 half of the weight free dim

F32 = mybir.dt.float32
BF16 = mybir.dt.bfloat16
ALU = mybir.AluOpType

# x row bands per ci-tile: band i covers padded rows [lo, hi)
XBANDS0 = [(0, 10), (10, 34), (34, 66)]   # sample 0: thin first band
XBANDS = [(0, 18), (18, 42), (42, 66)]    # prefetched samples
N_WARM = 44                               # HAM warm-up matmuls


def _split_multi_waits(bir: dict) -> int:
    """Hoist all but one sync wait from every instruction onto single-wait
    EventSemaphore carriers inserted immediately before it (same engine)."""
    ctr = 0
    for fn in bir.get("functions", []):
        for blk in fn.get("blocks", []):
            insts = blk.get("instructions", [])
            if not any(
                len(((i.get("sync_info") or {}).get("on_wait") or [])) > 1
                for i in insts
            ):
                continue
            new_insts = []
            for inst in insts:
                si = inst.get("sync_info")
                ow = (si or {}).get("on_wait") or []
                if len(ow) > 1:
                    for w in ow[:-1]:
                        ctr += 1
                        new_insts.append({
                            "debug": inst.get("debug", 0),
                            "engine": inst["engine"],
                            "ins": [],
                            "outs": [],
                            "name": f"waitsplit-{ctr}",
                            "opcode": "EventSemaphore",
                            "sync_info": {"on_update": [], "on_wait": [w]},
                        })
                    si["on_wait"] = [ow[-1]]
                new_insts.append(inst)
            blk["instructions"] = new_insts
    return ctr


def _dedup_ldweights(bir: dict) -> int:
    """The scheduled BIR carries one Ldweights per Matmult. When consecutive
    Ldweights load byte-identical operands (tap-major half-sweeps: 4 matmuls
    per weight) the repeats are redundant — the PE array still holds those
    weights. Delete them (only if they carry no sync_info), saving the ~2.5ns
    NX dispatch each in the PE stream."""
    import json as _json
    dropped = 0
    for fn in bir.get("functions", []):
        for blk in fn.get("blocks", []):
            insts = blk.get("instructions", [])
            prev_key = None
            keep = []
            for inst in insts:
                op = inst.get("opcode")
                if op == "Ldweights":
                    si = inst.get("sync_info") or {}
                    clean = not (si.get("on_wait") or si.get("on_update"))
                    key = _json.dumps(inst.get("ins"), sort_keys=True)
                    if clean and key == prev_key:
                        dropped += 1
                        continue
                    prev_key = key
                elif op not in ("Matmult", "EventSemaphore") \
                        and inst.get("engine") == "PE":
                    # a non-matmul PE instruction invalidates reuse tracking
                    # conservatively; other engines can't touch the PE array
                    prev_key = None
                keep.append(inst)
            blk["instructions"] = keep
    return dropped


def _batch_pe_incs(bir: dict) -> int:
    """Matmults complete in PC order, so the per-MM +1 on the global PE
    progress semaphore can be batched: keep an increment only on MMs whose
    cumulative count equals some waiter's threshold (and the last MM),
    carrying the accumulated value. Every waiter wakes on the exact same MM
    as before; ~1k EVT_SEM register writes disappear from the PE stream."""
    # find the semaphore id that (only) Matmults bump by 1
    upd_by = {}
    for fn in bir.get("functions", []):
        for blk in fn.get("blocks", []):
            for inst in blk.get("instructions", []):
                for u in ((inst.get("sync_info") or {}).get("on_update") or []):
                    if u.get("sync_type") == "semaphore":
                        key = u["id"]
                        upd_by.setdefault(key, []).append(
                            (inst["opcode"], u.get("update_mode"), u.get("update_value")))
    target = None
    for sid, ups in upd_by.items():
        if len(ups) > 500 and all(
                op == "Matmult" and mode == "sem-inc" and val == 1
                for op, mode, val in ups):
            target = sid
            break
    if target is None:
        return 0
    # collect all wait thresholds on that semaphore
    thresholds = set()
    for fn in bir.get("functions", []):
        for blk in fn.get("blocks", []):
            for inst in blk.get("instructions", []):
                for w in ((inst.get("sync_info") or {}).get("on_wait") or []):
                    if w.get("sync_type") == "semaphore" and w["id"] == target:
                        if w.get("wait_mode") != "sem-ge-imm":
                            return 0  # unexpected mode; bail
                        thresholds.add(w["wait_value"])
    # keep a single +1 inc only at threshold MMs (and the last), renumber
    # every waiter's threshold to its rank in the kept sequence — identical
    # wake MMs, update_value stays 1 everywhere
    events = []
    for fn in bir.get("functions", []):
        for blk in fn.get("blocks", []):
            for inst in blk.get("instructions", []):
                si = inst.get("sync_info") or {}
                for u in (si.get("on_update") or []):
                    if u.get("sync_type") == "semaphore" and u["id"] == target:
                        events.append((inst, u))
    removed = 0
    c = 0
    rank = {}
    for idx, (inst, u) in enumerate(events):
        c += 1
        if c in thresholds or idx == len(events) - 1:
            rank[c] = len(rank) + 1
        else:
            si = inst["sync_info"]
            si["on_update"] = [x for x in si["on_update"] if x is not u]
            removed += 1
    for fn in bir.get("functions", []):
        for blk in fn.get("blocks", []):
            for inst in blk.get("instructions", []):
                for w in ((inst.get("sync_info") or {}).get("on_wait") or []):
                    if w.get("sync_type") == "semaphore" and w["id"] == target:
                        w["wait_value"] = rank[w["wait_value"]]
    return removed


class _WaitSplitBass(bass.Bass):
    def to_json_bytes(self) -> bytes:
        raw = super().to_json_bytes()
        bir = json.loads(raw)
        changed = _split_multi_waits(bir)
        changed += _batch_pe_incs(bir)
        changed += _dedup_ldweights(bir)
        if changed:
            return json.dumps(bir).encode()
        return raw


def _pbcast(ap, n):
    """Manual 0-step partition broadcast AP (DMA-only; engines reject it)."""
    return bass.AP(tensor=ap.tensor, offset=ap.offset,
                   ap=[[0, n]] + [list(d) for d in ap.ap[1:]])


def _build_program() -> bass.Bass:
    nc = _WaitSplitBass("TRN2", target_bir_lowering=False, debug=False)

    # Sample 0: plain padded image (2.2MB — fits the head DMA budget, matmuls
    # use a row-strided moving AP at ~218ns). Samples 1-3: host pre-shifted
    # copies per horizontal tap dx (x3[b-1, dx, ci, r, 0:64] =
    # padded_x[b, ci, r, dx:dx+64]) so every moving operand is a fully
    # CONTIGUOUS [128, 512] slice (~216ns/MM); their 3x traffic is prefetched
    # far ahead on the otherwise-idle sync queue.
    x0_d = nc.dram_tensor("x0", [CIN, SPP, SPP], BF16, kind="ExternalInput")
    x3_d = nc.dram_tensor("x3", [BPC - 1, 3, CIN, SPP, SP], BF16,
                          kind="ExternalInput")
    wsc_d = nc.dram_tensor("wsc", [BPC, CIN, CKK], BF16, kind="ExternalInput")
    spmd_d = nc.dram_tensor("spmd", [BPC, YX], BF16, kind="ExternalInput")
    out_d = nc.dram_tensor("out", [BPC, COUT, SP, SP], F32, kind="ExternalOutput")

    with tile.TileContext(nc) as tc:
        with tc.tile_pool(name="const", bufs=1) as cpool:

            warm_w = cpool.tile([128, 128], BF16, name="warm_w")

            _stack = ExitStack()
            xppool = _stack.enter_context(tc.tile_pool(name="xp", bufs=2))
            cpsum = _stack.enter_context(tc.tile_pool(name="cps", bufs=8, space="PSUM"))
            wscpool = _stack.enter_context(tc.tile_pool(name="wsc", bufs=2))
            opool = _stack.enter_context(tc.tile_pool(name="ot", bufs=8))
            smpool = _stack.enter_context(tc.tile_pool(name="smb", bufs=32))

            # ---------- warm-up matmuls: keep the PE busy from t~7us so the
            # HAM clock gate is at K=8/8 (2.4 GHz) before real matmuls.
            # Scratch shares the 8-deep conv PSUM rotation (never read). ----
            nc.vector.memset(warm_w, 0.0)
            scratch = cpsum.tile([128, 512], F32, name="scratch", tag="ps")
            wsc_tiles = [None] * BPC
            ws0 = [
                wscpool.tile([128, CKK], BF16, name=f"wsc{k}_0", tag=f"wsc{k}")
                for k in range(2)
            ]
            wsc_tiles[0] = ws0
            for i in range(N_WARM):
                nc.tensor.matmul(scratch[:, 0:128], warm_w, warm_w,
                                 start=True, stop=True)

            # ---------- wsc loads (scalar queue; sample 0 m-split) ----------
            # ws[1]'s m=0 half is NOT loaded here: as the second 295KB on the
            # slow-ramping scalar queue it lands ~1.3us after n0's k=1 matmuls
            # need it (the recurring ~14.3us PE gap). It rides the sync queue
            # instead, between the thin x band and the (10,34) bands.
            def load_wsc(b, split=False):
                if split:
                    ws = wsc_tiles[b]
                    nc.scalar.dma_start(
                        out=ws[0][:, 0:HALF],
                        in_=wsc_d.ap()[b, 0:128, 0:HALF])
                    for k in range(2):
                        nc.scalar.dma_start(
                            out=ws[k][:, HALF:CKK],
                            in_=wsc_d.ap()[b, k * 128:(k + 1) * 128, HALF:CKK])
                else:
                    ws = [
                        wscpool.tile([128, CKK], BF16, name=f"wsc{k}_{b}", tag=f"wsc{k}")
                        for k in range(2)
                    ]
                    for k in range(2):
                        nc.scalar.dma_start(
                            out=ws[k],
                            in_=wsc_d.ap()[b, k * 128:(k + 1) * 128, :])
                    wsc_tiles[b] = ws

            smb_tiles = [[None] * 8 for _ in range(BPC)]

            def load_smb(b):
                for n in range(8):
                    t = smpool.tile([128, 512], BF16, name=f"smb_{b}_{n}", tag="smb")
                    nc.scalar.dma_start(
                        out=t,
                        in_=_pbcast(spmd_d.ap()[b:b + 1, n * 512:(n + 1) * 512], 128),
                    )
                    smb_tiles[b][n] = t

            load_wsc(0, split=True)
            load_smb(0)

            # ---------- x band loads (sync queue) ----------
            xp0 = [
                xppool.tile([128, SPP * SPP], BF16, name=f"xq{k}_0",
                            tag=f"xq{k}", bufs=1)
                for k in range(2)
            ]
            xp_tiles = [xp0] + [None] * (BPC - 1)

            for bi in range(len(XBANDS0)):
                lo, hi = XBANDS0[bi]
                for k in range(2):
                    nc.sync.dma_start(
                        out=xp0[k][:, lo * SPP:hi * SPP],
                        in_=x0_d.ap()[k * 128:(k + 1) * 128, lo:hi, :]
                            .rearrange("p r c -> p (r c)"),
                    )
                if bi == 0:
                    # ws[1] m=0 half here: after the thin band (first conv MM
                    # unaffected), before (10,34) (which has ~1.5us of slack)
                    nc.sync.dma_start(
                        out=wsc_tiles[0][1][:, 0:HALF],
                        in_=wsc_d.ap()[0, 128:256, 0:HALF])

            def load_band(xp, b, bi, k, dx, bands=XBANDS):
                lo, hi = bands[bi]
                nc.sync.dma_start(
                    out=xp[k][dx][:, lo * SP:hi * SP],
                    in_=x3_d.ap()[b - 1, dx, k * 128:(k + 1) * 128, lo:hi, :]
                        .rearrange("p r c -> p (r c)"),
                )

            def prefetch_xp(b):
                xp = [
                    [xppool.tile([128, SPP * SP], BF16,
                                 name=f"xp{k}{dx}_{b}", tag=f"xp{k}{dx}")
                     for dx in range(3)]
                    for k in range(2)
                ]
                xp_tiles[b] = xp
                return xp

            # ---------- per-sample conv pipeline ----------
            for b in range(BPC):
                wsc = wsc_tiles[b]
                xp = xp_tiles[b]
                prefetched = False

                for m in range(2):
                    for n in range(8):
                        last = (b == BPC - 1 and m == 1 and n == 7)
                        # very last chunk: two 256-col accumulation groups so
                        # the first half's epilogue+store overlap the second
                        # half's matmuls, shortening the serial tail
                        halves = (
                            [(0, 512)] if not last else [(0, 256), (256, 256)]
                        )
                        pss = []
                        for hi, (c0, cw) in enumerate(halves):
                            ps = cpsum.tile([128, 512], F32,
                                            name=f"ps_{b}_{m}_{n}_{hi}", tag="ps")
                            pss.append(ps)
                            i = 0
                            for k in range(2):
                                wv = wsc[k].rearrange("p (co kk) -> p co kk", kk=9)
                                for s in range(9):
                                    dy, dx = s // 3, s % 3
                                    if b == 0:
                                        xpv = xp[k].rearrange(
                                            "p (r c) -> p r c", c=SPP)
                                        rhs = xpv[:, n * 8 + dy:n * 8 + dy + 8,
                                                  dx:dx + SP]
                                    else:
                                        base = (n * 8 + dy) * SP + c0
                                        rhs = xp[k][dx][:, base:base + cw]
                                    nc.tensor.matmul(
                                        ps[:, 0:cw],
                                        wv[:, m * 128:(m + 1) * 128, s],
                                        rhs,
                                        start=(i == 0), stop=(i == 17),
                                    )
                                    i += 1
                            # epilogue for this half; stores ride the scalar
                            # queue (sync is dedicated to x loads), except the
                            # very last piece which takes the then-idle sync
                            eng = nc.sync if (last and hi == 1) else nc.scalar
                            ot = opool.tile([128, 512], F32,
                                            name=f"ot_{b}_{m}_{n}_{hi}", tag="ot")
                            nc.vector.tensor_tensor(
                                out=ot[:, 0:cw], in0=ps[:, 0:cw],
                                in1=smb_tiles[b][n][:, c0:c0 + cw], op=ALU.mult,
                            )
                            r0 = n * 8 + c0 // SP
                            eng.dma_start(
                                out=out_d.ap()[b, m * 128:(m + 1) * 128,
                                               r0:r0 + cw // SP, :],
                                in_=ot[:, 0:cw]
                                    .rearrange("p (r c) -> p r c", c=SP),
                            )
                        # prefetch next sample's weights + spmap broadcasts
                        if m == 0 and n == 6 and b + 1 < BPC:
                            load_wsc(b + 1)
                            load_smb(b + 1)
                        # prefetch next sample's image in m=1
                        if m == 1 and 1 <= n <= 3 and b + 1 < BPC:
                            if not prefetched:
                                xpn = prefetch_xp(b + 1)
                                prefetched = True
                            for k in range(2):
                                for dx in range(3):
                                    load_band(xpn, b + 1, n - 1, k, dx)
            _stack.close()
    return nc


_prog_cache = {}


def _get_program() -> bass.Bass:
    if "nc" not in _prog_cache:
        _prog_cache["nc"] = _build_program()
    return _prog_cache["nc"]


def _make_in_maps(inputs):
    x = np.asarray(inputs["x"], dtype=np.float32)
    xpad = np.pad(x, ((0, 0), (0, 0), (1, 1), (1, 1))).astype(ml_dtypes.bfloat16)
    # pre-shifted copies per horizontal tap: x3[b, dx] = xpad[..., dx:dx+64]
    # (used for samples 1-3 of each core; sample 0 gets the plain image)
    x3 = np.empty((B, 3, CIN, SPP, SP), dtype=ml_dtypes.bfloat16)
    for dx in range(3):
        x3[:, dx] = xpad[:, :, :, dx:dx + SP]
    style_in = np.asarray(inputs["style_in"], dtype=np.float64)
    weight = np.asarray(inputs["weight"], dtype=np.float64)
    mod_w = np.asarray(inputs["mod_w"], dtype=np.float64)
    mod_b = np.asarray(inputs["mod_b"], dtype=np.float64)
    sp_w = np.asarray(inputs["sp_w"], dtype=np.float64)
    sp_b = np.asarray(inputs["sp_b"], dtype=np.float64)

    # ---- host-side small math (float64, a few MFLOPs total) ----
    style_chan, style_spatial = style_in[:, :256], style_in[:, 256:]
    style = style_chan @ (mod_w * LS).T + mod_b                 # [B, CIN]
    # demod via S2q[co,ci] = sum_kk weight^2 (exact same sum as reference)
    w0 = weight[0]                                              # [COUT,CIN,3,3]
    s2q = np.sum(w0 * w0, axis=(2, 3))                          # [COUT, CIN]
    demodsq = (CS * CS) * (style * style) @ s2q.T               # [B, COUT]
    demod = 1.0 / np.sqrt(demodsq + EPS)
    sp = style_spatial @ (sp_w * LS).T + sp_b                   # [B, YX]
    demod_sp = np.sqrt(YX / np.sum(sp * sp, axis=1) + EPS)      # [B]
    dcol = CS * demod * demod_sp[:, None]                       # [B, COUT]
    spmd = sp.astype(ml_dtypes.bfloat16)                        # [B, YX]

    # per-sample folded weights: [B, CIN, COUT*KK] bf16
    wT = np.ascontiguousarray(w0.transpose(1, 0, 2, 3)).reshape(CIN, COUT, K * K)
    wsc = (wT[None].astype(np.float32)
           * style.astype(np.float32)[:, :, None, None]
           * dcol.astype(np.float32)[:, None, :, None])         # [B,CIN,COUT,KK]
    wsc = wsc.reshape(B, CIN, CKK).astype(ml_dtypes.bfloat16)

    in_maps = []
    for c in range(N_CORES):
        sl = slice(c * BPC, (c + 1) * BPC)
        in_maps.append({
            "x0": np.ascontiguousarray(xpad[c * BPC]),
            "x3": np.ascontiguousarray(x3[c * BPC + 1:(c + 1) * BPC]),
            "wsc": np.ascontiguousarray(wsc[sl]),
            "spmd": np.ascontiguousarray(spmd[sl]),
        })
    return in_maps


def _run(inputs, trace=False):
    nc = _get_program()
    in_maps = _make_in_maps(inputs)
    res = run_bass_kernel_spmd(nc, in_maps, core_ids=list(range(N_CORES)), trace=trace)
    out = np.concatenate([res.results[c]["out"] for c in range(N_CORES)], axis=0)
    return out, res


def kernel(**inputs) -> np.ndarray:
    out, _ = _run(inputs, trace=False)
    return out


# revision 63
# speedup vs baseline: 1.0054x; 1.0054x over previous
"""Trainium2 Bass kernel for nn_ChannelSpatialModulatedConv2d.

Data-parallel over batch across 8 NeuronCores (4 samples each).

All batch-parallel small math (EqualLinear style, weight demod, spatial map
+ its demod) runs on HOST in float64 — a few MFLOPs. The per-sample
modulated+demodulated weights are also folded on host:

  wsc[b, ci, co*kk] = weight[co,ci,kk] * style[b,ci] * (CS*demod[b,co]*demod_sp[b])

so the device kernel is a pure bf16 conv stream:

  per (b, m, n) output tile [128co x 512yx]: 18 accumulating bf16 matmuls
     (2 ci-tiles x 9 taps) over a zero-padded 66x66 image, shifted-window APs
  epilogue (DVE): out = psum * spmap[yx]   (spmap = raw sp map, bf16,
     broadcast 1->128 partitions by DMA)

PE stream = ~40 warm-up matmuls (HAM K=8/8 by the time real MMs start)
followed by 1152 conv matmuls at the N=512 streaming roofline. The only
other device compute is the 64 epilogue multiplies on DVE.

DMA: sync(HWDGE) = x bands + outs (first x band thinned to rows 0-10);
scalar(HWDGE) = wsc tiles (sample 0 split into m-halves so conv starts
ASAP) + spmap broadcasts (sample 0 up-front, rest staggered per sample).

The baked walrus build only supports ONE sync wait per instruction, so the
Bass subclass rewrites the scheduled BIR JSON, hoisting extra waits onto
single-wait EventSemaphore carriers inserted before the instruction (same
engine => identical blocking semantics).
"""

import json
import sys
from contextlib import ExitStack

for _p in ("/opt/pypackages", "/opt/trn_rl_repo"):
    if _p not in sys.path:
        sys.path.insert(0, _p)

import ml_dtypes
import numpy as np

import concourse.bass as bass
import concourse.mybir as mybir
import concourse.tile as tile
from concourse.bass_utils import run_bass_kernel_spmd

# Problem constants (hardcoded per harness contract)
B, CIN, COUT, K = 32, 256, 256, 3
STYLE_DIM, SP = 512, 64
EPS = 1e-6
LS = 1.0 / (STYLE_DIM // 2) ** 0.5      # EqualLinear scale = 1/16
CS = 1.0 / (CIN * K * K) ** 0.5         # conv fan-in scale = 1/48
N_CORES = 8
BPC = B // N_CORES                      # samples per core = 4
SPP = SP + 2                            # padded image dim = 66
CKK = COUT * K * K                      # 2304 free columns in weight layout
YX = SP * SP                            # 4096 spatial positions
HALF = (CKK // 2)                       # m=0 half of the weight free dim

F32 = mybir.dt.float32
BF16 = mybir.dt.bfloat16
ALU = mybir.AluOpType

# x row bands per ci-tile: band i covers padded rows [lo, hi)
XBANDS0 = [(0, 10), (10, 34), (34, 66)]   # sample 0: thin first band
XBANDS = [(0, 18), (18, 42), (42, 66)]    # prefetched samples
N_WARM = 44                               # HAM warm-up matmuls


def _split_multi_waits(bir: dict) -> int:
    """Hoist all but one sync wait from every instruction onto single-wait
    EventSemaphore carriers inserted immediately before it (same engine)."""
    ctr = 0
    for fn in bir.get("functions", []):
        for blk in fn.get("blocks", []):
            insts = blk.get("instructions", [])
            if not any(
                len(((i.get("sync_info") or {}).get("on_wait") or [])) > 1
                for i in insts
            ):
                continue
            new_insts = []
            for inst in insts:
                si = inst.get("sync_info")
                ow = (si or {}).get("on_wait") or []
                if len(ow) > 1:
                    for w in ow[:-1]:
                        ctr += 1
                        new_insts.append({
                            "debug": inst.get("debug", 0),
                            "engine": inst["engine"],
                            "ins": [],
                            "outs": [],
                            "name": f"waitsplit-{ctr}",
                            "opcode": "EventSemaphore",
                            "sync_info": {"on_update": [], "on_wait": [w]},
                        })
                    si["on_wait"] = [ow[-1]]
                new_insts.append(inst)
            blk["instructions"] = new_insts
    return ctr


def _dedup_ldweights(bir: dict) -> int:
    """The scheduled BIR carries one Ldweights per Matmult. When consecutive
    Ldweights load byte-identical operands (tap-major half-sweeps: 4 matmuls
    per weight) the repeats are redundant — the PE array still holds those
    weights. Delete them (only if they carry no sync_info), saving the ~2.5ns
    NX dispatch each in the PE stream."""
    import json as _json
    dropped = 0
    for fn in bir.get("functions", []):
        for blk in fn.get("blocks", []):
            insts = blk.get("instructions", [])
            prev_key = None
            keep = []
            for inst in insts:
                op = inst.get("opcode")
                if op == "Ldweights":
                    si = inst.get("sync_info") or {}
                    clean = not (si.get("on_wait") or si.get("on_update"))
                    key = _json.dumps(inst.get("ins"), sort_keys=True)
                    if clean and key == prev_key:
                        dropped += 1
                        continue
                    prev_key = key
                elif op not in ("Matmult", "EventSemaphore") \
                        and inst.get("engine") == "PE":
                    # a non-matmul PE instruction invalidates reuse tracking
                    # conservatively; other engines can't touch the PE array
                    prev_key = None
                keep.append(inst)
            blk["instructions"] = keep
    return dropped


def _batch_pe_incs(bir: dict) -> int:
    """Matmults complete in PC order, so the per-MM +1 on the global PE
    progress semaphore can be batched: keep an increment only on MMs whose
    cumulative count equals some waiter's threshold (and the last MM),
    carrying the accumulated value. Every waiter wakes on the exact same MM
    as before; ~1k EVT_SEM register writes disappear from the PE stream."""
    # find the semaphore id that (only) Matmults bump by 1
    upd_by = {}
    for fn in bir.get("functions", []):
        for blk in fn.get("blocks", []):
            for inst in blk.get("instructions", []):
                for u in ((inst.get("sync_info") or {}).get("on_update") or []):
                    if u.get("sync_type") == "semaphore":
                        key = u["id"]
                        upd_by.setdefault(key, []).append(
                            (inst["opcode"], u.get("update_mode"), u.get("update_value")))
    target = None
    for sid, ups in upd_by.items():
        if len(ups) > 500 and all(
                op == "Matmult" and mode == "sem-inc" and val == 1
                for op, mode, val in ups):
            target = sid
            break
    if target is None:
        return 0
    # collect all wait thresholds on that semaphore
    thresholds = set()
    for fn in bir.get("functions", []):
        for blk in fn.get("blocks", []):
            for inst in blk.get("instructions", []):
                for w in ((inst.get("sync_info") or {}).get("on_wait") or []):
                    if w.get("sync_type") == "semaphore" and w["id"] == target:
                        if w.get("wait_mode") != "sem-ge-imm":
                            return 0  # unexpected mode; bail
                        thresholds.add(w["wait_value"])
    # keep a single +1 inc only at threshold MMs (and the last), renumber
    # every waiter's threshold to its rank in the kept sequence — identical
    # wake MMs, update_value stays 1 everywhere
    events = []
    for fn in bir.get("functions", []):
        for blk in fn.get("blocks", []):
            for inst in blk.get("instructions", []):
                si = inst.get("sync_info") or {}
                for u in (si.get("on_update") or []):
                    if u.get("sync_type") == "semaphore" and u["id"] == target:
                        events.append((inst, u))
    removed = 0
    c = 0
    rank = {}
    for idx, (inst, u) in enumerate(events):
        c += 1
        if c in thresholds or idx == len(events) - 1:
            rank[c] = len(rank) + 1
        else:
            si = inst["sync_info"]
            si["on_update"] = [x for x in si["on_update"] if x is not u]
            removed += 1
    for fn in bir.get("functions", []):
        for blk in fn.get("blocks", []):
            for inst in blk.get("instructions", []):
                for w in ((inst.get("sync_info") or {}).get("on_wait") or []):
                    if w.get("sync_type") == "semaphore" and w["id"] == target:
                        w["wait_value"] = rank[w["wait_value"]]
    return removed


class _WaitSplitBass(bass.Bass):
    def to_json_bytes(self) -> bytes:
        raw = super().to_json_bytes()
        bir = json.loads(raw)
        changed = _split_multi_waits(bir)
        changed += _batch_pe_incs(bir)
        changed += _dedup_ldweights(bir)
        if changed:
            return json.dumps(bir).encode()
        return raw


def _pbcast(ap, n):
    """Manual 0-step partition broadcast AP (DMA-only; engines reject it)."""
    return bass.AP(tensor=ap.tensor, offset=ap.offset,
                   ap=[[0, n]] + [list(d) for d in ap.ap[1:]])


def _build_program() -> bass.Bass:
    nc = _WaitSplitBass("TRN2", target_bir_lowering=False, debug=False)

    # Sample 0: plain padded image (2.2MB — fits the head DMA budget, matmuls
    # use a row-strided moving AP at ~218ns). Samples 1-3: host pre-shifted
    # copies per horizontal tap dx (x3[b-1, dx, ci, r, 0:64] =
    # padded_x[b, ci, r, dx:dx+64]) so every moving operand is a fully
    # CONTIGUOUS [128, 512] slice (~216ns/MM); their 3x traffic is prefetched
    # far ahead on the otherwise-idle sync queue.
    x0_d = nc.dram_tensor("x0", [CIN, SPP, SPP], BF16, kind="ExternalInput")
    x3_d = nc.dram_tensor("x3", [BPC - 1, 3, CIN, SPP, SP], BF16,
                          kind="ExternalInput")
    wsc_d = nc.dram_tensor("wsc", [BPC, CIN, CKK], BF16, kind="ExternalInput")
    spmd_d = nc.dram_tensor("spmd", [BPC, YX], BF16, kind="ExternalInput")
    out_d = nc.dram_tensor("out", [BPC, COUT, SP, SP], F32, kind="ExternalOutput")

    with tile.TileContext(nc) as tc:
        with tc.tile_pool(name="const", bufs=1) as cpool:

            warm_w = cpool.tile([128, 128], BF16, name="warm_w")

            _stack = ExitStack()
            xppool = _stack.enter_context(tc.tile_pool(name="xp", bufs=2))
            cpsum = _stack.enter_context(tc.tile_pool(name="cps", bufs=8, space="PSUM"))
            wscpool = _stack.enter_context(tc.tile_pool(name="wsc", bufs=2))
            opool = _stack.enter_context(tc.tile_pool(name="ot", bufs=8))
            smpool = _stack.enter_context(tc.tile_pool(name="smb", bufs=32))

            # ---------- warm-up matmuls: keep the PE busy from t~7us so the
            # HAM clock gate is at K=8/8 (2.4 GHz) before real matmuls.
            # Scratch shares the 8-deep conv PSUM rotation (never read). ----
            nc.vector.memset(warm_w, 0.0)
            scratch = cpsum.tile([128, 512], F32, name="scratch", tag="ps")
            wsc_tiles = [None] * BPC
            ws0 = [
                wscpool.tile([128, CKK], BF16, name=f"wsc{k}_0", tag=f"wsc{k}")
                for k in range(2)
            ]
            wsc_tiles[0] = ws0
            for i in range(N_WARM):
                nc.tensor.matmul(scratch[:, 0:128], warm_w, warm_w,
                                 start=True, stop=True)

            # ---------- wsc loads (scalar queue; sample 0 m-split) ----------
            def load_wsc(b, split=False):
                if split:
                    # all on scalar queue (x bands own sync); m=0 halves first
                    ws = wsc_tiles[b]
                    for k in range(2):
                        nc.scalar.dma_start(
                            out=ws[k][:, 0:HALF],
                            in_=wsc_d.ap()[b, k * 128:(k + 1) * 128, 0:HALF])
                    for k in range(2):
                        nc.scalar.dma_start(
                            out=ws[k][:, HALF:CKK],
                            in_=wsc_d.ap()[b, k * 128:(k + 1) * 128, HALF:CKK])
                else:
                    ws = [
                        wscpool.tile([128, CKK], BF16, name=f"wsc{k}_{b}", tag=f"wsc{k}")
                        for k in range(2)
                    ]
                    for k in range(2):
                        nc.scalar.dma_start(
                            out=ws[k],
                            in_=wsc_d.ap()[b, k * 128:(k + 1) * 128, :])
                    wsc_tiles[b] = ws

            smb_tiles = [[None] * 8 for _ in range(BPC)]

            def load_smb(b):
                for n in range(8):
                    t = smpool.tile([128, 512], BF16, name=f"smb_{b}_{n}", tag="smb")
                    nc.scalar.dma_start(
                        out=t,
                        in_=_pbcast(spmd_d.ap()[b:b + 1, n * 512:(n + 1) * 512], 128),
                    )
                    smb_tiles[b][n] = t

            load_wsc(0, split=True)
            load_smb(0)

            # ---------- x band loads (sync queue) ----------
            xp0 = [
                xppool.tile([128, SPP * SPP], BF16, name=f"xq{k}_0",
                            tag=f"xq{k}", bufs=1)
                for k in range(2)
            ]
            xp_tiles = [xp0] + [None] * (BPC - 1)

            # k-outer order: the two-pass b0-m0 schedule consumes all k=0
            # bands first; k=1 bands are not needed until ~27us
            for k in range(2):
                for bi in range(len(XBANDS0)):
                    lo, hi = XBANDS0[bi]
                    nc.sync.dma_start(
                        out=xp0[k][:, lo * SPP:hi * SPP],
                        in_=x0_d.ap()[k * 128:(k + 1) * 128, lo:hi, :]
                            .rearrange("p r c -> p (r c)"),
                    )

            def load_band(xp, b, bi, k, dx, bands=XBANDS):
                lo, hi = bands[bi]
                nc.sync.dma_start(
                    out=xp[k][dx][:, lo * SP:hi * SP],
                    in_=x3_d.ap()[b - 1, dx, k * 128:(k + 1) * 128, lo:hi, :]
                        .rearrange("p r c -> p (r c)"),
                )

            def prefetch_xp(b):
                xp = [
                    [xppool.tile([128, SPP * SP], BF16,
                                 name=f"xp{k}{dx}_{b}", tag=f"xp{k}{dx}")
                     for dx in range(3)]
                    for k in range(2)
                ]
                xp_tiles[b] = xp
                return xp

            # ---------- per-sample conv pipeline ----------
            for b in range(BPC):
                wsc = wsc_tiles[b]
                xp = xp_tiles[b]
                prefetched = False

                for m in range(2):
                    if b == 0 and m == 0:
                        # two-pass k-split: all k=0 taps for the 8 groups
                        # first (only ws[0]-half + k=0 bands needed early),
                        # then the k=1 pass from ~27us when its data has
                        # long arrived — removes the recurring head-DMA gap
                        pss0 = [
                            cpsum.tile([128, 512], F32,
                                       name=f"ps_0_0_{n}", tag="ps")
                            for n in range(8)
                        ]
                        for k in range(2):
                            wv = wsc[k].rearrange("p (co kk) -> p co kk", kk=9)
                            xpv = xp[k].rearrange("p (r c) -> p r c", c=SPP)
                            for n in range(8):
                                for s in range(9):
                                    dy, dx = s // 3, s % 3
                                    nc.tensor.matmul(
                                        pss0[n],
                                        wv[:, 0:128, s],
                                        xpv[:, n * 8 + dy:n * 8 + dy + 8,
                                            dx:dx + SP],
                                        start=(k == 0 and s == 0),
                                        stop=(k == 1 and s == 8),
                                    )
                                if k == 1:
                                    ot = opool.tile([128, 512], F32,
                                                    name=f"ot_0_0_{n}", tag="ot")
                                    nc.vector.tensor_tensor(
                                        out=ot, in0=pss0[n],
                                        in1=smb_tiles[0][n], op=ALU.mult,
                                    )
                                    nc.scalar.dma_start(
                                        out=out_d.ap()[0, 0:128,
                                                       n * 8:(n + 1) * 8, :],
                                        in_=ot.rearrange("p (r c) -> p r c",
                                                         c=SP),
                                    )
                                    if n == 6:
                                        load_wsc(1)
                                        load_smb(1)
                        continue
                    for n in range(8):
                        last = (b == BPC - 1 and m == 1 and n == 7)
                        # very last chunk: two 256-col accumulation groups so
                        # the first half's epilogue+store overlap the second
                        # half's matmuls, shortening the serial tail
                        halves = (
                            [(0, 512)] if not last else [(0, 256), (256, 256)]
                        )
                        pss = []
                        for hi, (c0, cw) in enumerate(halves):
                            ps = cpsum.tile([128, 512], F32,
                                            name=f"ps_{b}_{m}_{n}_{hi}", tag="ps")
                            pss.append(ps)
                            i = 0
                            for k in range(2):
                                wv = wsc[k].rearrange("p (co kk) -> p co kk", kk=9)
                                for s in range(9):
                                    dy, dx = s // 3, s % 3
                                    if b == 0:
                                        xpv = xp[k].rearrange(
                                            "p (r c) -> p r c", c=SPP)
                                        rhs = xpv[:, n * 8 + dy:n * 8 + dy + 8,
                                                  dx:dx + SP]
                                    else:
                                        base = (n * 8 + dy) * SP + c0
                                        rhs = xp[k][dx][:, base:base + cw]
                                    nc.tensor.matmul(
                                        ps[:, 0:cw],
                                        wv[:, m * 128:(m + 1) * 128, s],
                                        rhs,
                                        start=(i == 0), stop=(i == 17),
                                    )
                                    i += 1
                            # epilogue for this half; stores ride the scalar
                            # queue (sync is dedicated to x loads), except the
                            # very last piece which takes the then-idle sync
                            eng = nc.sync if (last and hi == 1) else nc.scalar
                            ot = opool.tile([128, 512], F32,
                                            name=f"ot_{b}_{m}_{n}_{hi}", tag="ot")
                            nc.vector.tensor_tensor(
                                out=ot[:, 0:cw], in0=ps[:, 0:cw],
                                in1=smb_tiles[b][n][:, c0:c0 + cw], op=ALU.mult,
                            )
                            r0 = n * 8 + c0 // SP
                            eng.dma_start(
                                out=out_d.ap()[b, m * 128:(m + 1) * 128,
                                               r0:r0 + cw // SP, :],
                                in_=ot[:, 0:cw]
                                    .rearrange("p (r c) -> p r c", c=SP),
                            )
                        # prefetch next sample's weights + spmap broadcasts
                        if m == 0 and n == 6 and b + 1 < BPC:
                            load_wsc(b + 1)
                            load_smb(b + 1)
                        # prefetch next sample's image in m=1
                        if m == 1 and 1 <= n <= 3 and b + 1 < BPC:
                            if not prefetched:
                                xpn = prefetch_xp(b + 1)
                                prefetched = True
                            for k in range(2):
                                for dx in range(3):
                                    load_band(xpn, b + 1, n - 1, k, dx)
            _stack.close()
    return nc


_prog_cache = {}


def _get_program() -> bass.Bass:
    if "nc" not in _prog_cache:
        _prog_cache["nc"] = _build_program()
    return _prog_cache["nc"]


def _make_in_maps(inputs):
    x = np.asarray(inputs["x"], dtype=np.float32)
    xpad = np.pad(x, ((0, 0), (0, 0), (1, 1), (1, 1))).astype(ml_dtypes.bfloat16)
    # pre-shifted copies per horizontal tap: x3[b, dx] = xpad[..., dx:dx+64]
    # (used for samples 1-3 of each core; sample 0 gets the plain image)
    x3 = np.empty((B, 3, CIN, SPP, SP), dtype=ml_dtypes.bfloat16)
    for dx in range(3):
        x3[:, dx] = xpad[:, :, :, dx:dx + SP]
    style_in = np.asarray(inputs["style_in"], dtype=np.float64)
    weight = np.asarray(inputs["weight"], dtype=np.float64)
    mod_w = np.asarray(inputs["mod_w"], dtype=np.float64)
    mod_b = np.asarray(inputs["mod_b"], dtype=np.float64)
    sp_w = np.asarray(inputs["sp_w"], dtype=np.float64)
    sp_b = np.asarray(inputs["sp_b"], dtype=np.float64)

    # ---- host-side small math (float64, a few MFLOPs total) ----
    style_chan, style_spatial = style_in[:, :256], style_in[:, 256:]
    style = style_chan @ (mod_w * LS).T + mod_b                 # [B, CIN]
    # demod via S2q[co,ci] = sum_kk weight^2 (exact same sum as reference)
    w0 = weight[0]                                              # [COUT,CIN,3,3]
    s2q = np.sum(w0 * w0, axis=(2, 3))                          # [COUT, CIN]
    demodsq = (CS * CS) * (style * style) @ s2q.T               # [B, COUT]
    demod = 1.0 / np.sqrt(demodsq + EPS)
    sp = style_spatial @ (sp_w * LS).T + sp_b                   # [B, YX]
    demod_sp = np.sqrt(YX / np.sum(sp * sp, axis=1) + EPS)      # [B]
    dcol = CS * demod * demod_sp[:, None]                       # [B, COUT]
    spmd = sp.astype(ml_dtypes.bfloat16)                        # [B, YX]

    # per-sample folded weights: [B, CIN, COUT*KK] bf16
    wT = np.ascontiguousarray(w0.transpose(1, 0, 2, 3)).reshape(CIN, COUT, K * K)
    wsc = (wT[None].astype(np.float32)
           * style.astype(np.float32)[:, :, None, None]
           * dcol.astype(np.float32)[:, None, :, None])         # [B,CIN,COUT,KK]
    wsc = wsc.reshape(B, CIN, CKK).astype(ml_dtypes.bfloat16)

    in_maps = []
    for c in range(N_CORES):
        sl = slice(c * BPC, (c + 1) * BPC)
        in_maps.append({
            "x0": np.ascontiguousarray(xpad[c * BPC]),
            "x3": np.ascontiguousarray(x3[c * BPC + 1:(c + 1) * BPC]),
            "wsc": np.ascontiguousarray(wsc[sl]),
            "spmd": np.ascontiguousarray(spmd[sl]),
        })
    return in_maps


def _run(inputs, trace=False):
    nc = _get_program()
    in_maps = _make_in_maps(inputs)
    res = run_bass_kernel_spmd(nc, in_maps, core_ids=list(range(N_CORES)), trace=trace)
    out = np.concatenate([res.results[c]["out"] for c in range(N_CORES)], axis=0)
    return out, res


def kernel(**inputs) -> np.ndarray:
    out, _ = _run(inputs, trace=False)
    return out
